# revision 14
# baseline (speedup 1.0000x reference)
"""Trainium2 Bass kernel for the MoE block (nn_MoEBlock_5592047420171).

Strategy: data-parallel over tokens across 8 NeuronCores (1024 tokens/core,
all weights replicated; no collectives).  Per core, layout A (d_ff on
partitions, tokens on the free dim):

  out[t,:] = v_t * (relu(base_t + delta_{e1(t),t}) + relu(base_t + delta_{e2(t),t}))
             @ wo^T + 2 v_t * bo
  base = hs wi^T + bi,  delta_e = (hs lA_e^T) lB_e^T,  v = top2 softmax mass

Key structure:
  * router computes P=exp(logits) in real fp32 (so top-2 selection matches the
    fp32 reference); the top-8 DVE sort gives max/second-max; one-hot masks of
    the first/second choice expert are built per token.
  * the 8 per-expert rank-16 lora paths collapse to TWO dense K=128 matmuls:
    tA (all experts' lora-A outputs, 8x16 rows) is masked per token by the
    first/second-choice one-hot (16-row groups), then multiplied by the
    concatenated lora-B.  No per-expert loop on the hot path.
  * base is computed once per tile and added into each choice's PSUM bank
    with an identity matmul; bi rides the PSUM->SBUF copy as an ACT bias.
  * val_sum v is pulled out of the expert sum (one broadcast multiply per
    tile); 2*v*bo rides the wo matmul as a K=1 rank-1 term.
  * all big matmuls are bf16 (fp32r measured at half the bf16 rate).
"""

import numpy as np
from contextlib import ExitStack

import concourse.bass as bass
import concourse.tile as tile
from concourse import bacc, mybir
from concourse.bass_utils import run_bass_kernel_spmd
from concourse.masks import make_identity

B, S, DM, FF, E, RK = 4, 2048, 1024, 4096, 8, 16
NCORES = 8
TOK = B * S            # 8192 tokens
T = TOK // NCORES      # 1024 tokens per core
TCH = T // 128         # 8 token chunks of 128
FCH = FF // 128        # 32 d_ff chunks of 128
TT = 512               # token tile width (free dim of big matmuls)
NTT = T // TT          # 2 token tiles
NKC = 9                # router contraction chunks: 8 x 128 d_model + bias
DCH = 8                # d_model chunks for the bf16 matmuls

F32 = mybir.dt.float32
BF16 = mybir.dt.bfloat16
AX = mybir.AxisListType
ALU = mybir.AluOpType
AF = mybir.ActivationFunctionType


def build_bass():
    nc = bacc.Bacc("TRN2", target_bir_lowering=False)

    hsR = nc.declare_dram_parameter("hsR", [128, NKC, T], F32, isOutput=False)
    hsB = nc.declare_dram_parameter("hsB", [128, DCH, T], BF16, isOutput=False)
    wiB = nc.declare_dram_parameter("wiB", [FCH, 128, DCH, 128], BF16, isOutput=False)
    biC = nc.declare_dram_parameter("biC", [128, FCH], F32, isOutput=False)
    lAc = nc.declare_dram_parameter("lAc", [128, DCH, 128], BF16, isOutput=False)
    lBc = nc.declare_dram_parameter("lBc", [FCH, 128, 128], BF16, isOutput=False)
    gw9 = nc.declare_dram_parameter("gw9", [128, NKC, 8], F32, isOutput=False)
    woB = nc.declare_dram_parameter("woB", [FCH, 128, DM], BF16, isOutput=False)
    bo2 = nc.declare_dram_parameter("bo2", [1, DM], BF16, isOutput=False)
    outT = nc.declare_dram_parameter("outT", [DM, T], F32, isOutput=True)

    hsR, hsB, wiB, biC, lAc, lBc, gw9, woB, bo2, outT = (
        h.ap() for h in (hsR, hsB, wiB, biC, lAc, lBc, gw9, woB, bo2, outT))

    with tile.TileContext(nc) as tc, ExitStack() as ctx:
        persist = ctx.enter_context(tc.tile_pool(name="persist", bufs=1))
        dram = ctx.enter_context(tc.tile_pool(name="dram", bufs=1, space="DRAM"))

        # ---- resident tensors ----
        hsR_sb = persist.tile([128, NKC, T], F32, tag="hsR")
        nc.sync.dma_start(out=hsR_sb, in_=hsR)
        hsB_sb = persist.tile([128, DCH, T], BF16, tag="hsB")
        nc.sync.dma_start(out=hsB_sb, in_=hsB)
        gw_sb = persist.tile([128, NKC, 8], F32, tag="gw")
        nc.sync.dma_start(out=gw_sb, in_=gw9)
        lA_sb = persist.tile([128, DCH, 128], BF16, tag="lA")
        nc.sync.dma_start(out=lA_sb, in_=lAc)
        bi_sb = persist.tile([128, FCH], F32, tag="bi")
        nc.sync.dma_start(out=bi_sb, in_=biC)
        bo2_sb = persist.tile([1, DM], BF16, tag="bo2")
        nc.sync.dma_start(out=bo2_sb, in_=bo2)
        ident = persist.tile([128, 128], F32, tag="ident")
        make_identity(nc, ident)
        identb = persist.tile([128, 128], BF16, tag="identb")
        nc.scalar.copy(out=identb, in_=ident)

        ohT_sb = persist.tile([16, T], BF16, tag="ohT")   # oh1 rows 0-7, oh2 8-15
        vT_sb = persist.tile([1, T], F32, tag="vT")       # val_sum row (f32)
        vTb_sb = persist.tile([1, T], BF16, tag="vTb")    # val_sum row (bf16)
        V_b = persist.tile([128, T], BF16, tag="Vb")      # val_sum bcast
        M1_sb = persist.tile([128, T], BF16, tag="M1")    # first-choice mask
        M2_sb = persist.tile([128, T], BF16, tag="M2")    # second-choice mask
        tA1_sb = persist.tile([128, T], BF16, tag="tA1")  # masked lora-A (1st)
        tA2_sb = persist.tile([128, T], BF16, tag="tA2")  # masked lora-A (2nd)
        H_sb = [persist.tile([128, T], BF16, tag=f"H{fc}", name=f"H{fc}")
                for fc in range(FCH)]
        vrow = dram.tile([1, T], BF16, tag="vrow")
        ohd = dram.tile([16, T], BF16, tag="ohd")

        # ---- phase 1: router ----
        with (
            tc.tile_pool(name="r_ps", bufs=2, space="PSUM") as r_ps,
            tc.tile_pool(name="tr_ps", bufs=2, space="PSUM") as tr_ps,
            tc.tile_pool(name="r_sb", bufs=3) as r_sb,
        ):
            for tch in range(TCH):
                tsl = slice(tch * 128, (tch + 1) * 128)
                lg = r_ps.tile([128, 8], F32, tag="lg")
                for ci in range(NKC):
                    nc.tensor.matmul(
                        lg,
                        lhsT=hsR_sb[:, ci, tsl],
                        rhs=gw_sb[:, ci, :],
                        start=(ci == 0), stop=(ci == NKC - 1),
                    )
                P = r_sb.tile([128, 8], F32, tag="P")
                nc.scalar.activation(P, lg, AF.Exp)
                top8 = r_sb.tile([128, 8], F32, tag="top8")
                nc.vector.max(out=top8, in_=P)
                sP = r_sb.tile([128, 1], F32, tag="sP")
                nc.vector.tensor_reduce(out=sP, in_=P, axis=AX.X, op=ALU.add)
                rv = r_sb.tile([128, 1], F32, tag="rv")
                nc.vector.reciprocal(rv, sP)
                # oh1 = (P == max); oh2 = (P >= m2) - oh1 ; v = (m1+m2)*rv
                ohb = r_sb.tile([128, 16], F32, tag="ohb")
                nc.vector.tensor_scalar(
                    out=ohb[:, 0:8], in0=P, scalar1=top8[:, 0:1], scalar2=None,
                    op0=ALU.is_equal,
                )
                nc.vector.scalar_tensor_tensor(
                    out=ohb[:, 8:16], in0=P, scalar=top8[:, 1:2],
                    in1=ohb[:, 0:8], op0=ALU.is_ge, op1=ALU.subtract,
                )
                vc = r_sb.tile([128, 1], F32, tag="vc")
                nc.vector.scalar_tensor_tensor(
                    out=vc, in0=top8[:, 0:1], scalar=top8[:, 1:2],
                    in1=rv, op0=ALU.add, op1=ALU.mult,
                )
                trp = tr_ps.tile([16, 128], F32, tag="trp")
                nc.tensor.transpose(trp, ohb, ident)
                trpv = tr_ps.tile([1, 128], F32, tag="trpv")
                nc.tensor.transpose(trpv, vc, ident)
                nc.scalar.copy(out=ohT_sb[:, tsl], in_=trp)
                nc.scalar.copy(out=vT_sb[:, tsl], in_=trpv[0:1, :])

        nc.scalar.copy(out=vTb_sb, in_=vT_sb)
        nc.sync.dma_start(out=vrow, in_=vTb_sb)
        nc.sync.dma_start(out=V_b, in_=vrow.to_broadcast([128, T]))
        nc.sync.dma_start(out=ohd, in_=ohT_sb)
        # expand one-hot rows to 16-row groups: M[16e+k, t] = oh[e, t]
        m1_src = bass.AP(tensor=ohd.tensor, offset=ohd.offset,
                         ap=[[T, 8], [0, 16], [1, T]])
        nc.sync.dma_start(out=M1_sb, in_=m1_src)
        m2_src = bass.AP(tensor=ohd.tensor, offset=ohd.offset + 8 * T,
                         ap=[[T, 8], [0, 16], [1, T]])
        nc.sync.dma_start(out=M2_sb, in_=m2_src)

        # ---- phase 2: lora-A projections + per-token choice masking ----
        with (
            tc.tile_pool(name="tA_ps", bufs=2, space="PSUM") as tA_ps,
            tc.tile_pool(name="tA_tmp", bufs=2) as tA_tmp,
        ):
            for tt in range(NTT):
                tsl = slice(tt * TT, (tt + 1) * TT)
                pta = tA_ps.tile([128, TT], F32, tag="pta")
                for ci in range(DCH):
                    nc.tensor.matmul(
                        pta,
                        lhsT=lA_sb[:, ci, :],
                        rhs=hsB_sb[:, ci, tsl],
                        start=(ci == 0), stop=(ci == DCH - 1),
                    )
                tAf = tA_tmp.tile([128, TT], BF16, tag="tAf")
                nc.scalar.copy(out=tAf, in_=pta)
                nc.vector.tensor_tensor(
                    out=tA1_sb[:, tsl], in0=tAf, in1=M1_sb[:, tsl], op=ALU.mult)
                nc.vector.tensor_tensor(
                    out=tA2_sb[:, tsl], in0=tAf, in1=M2_sb[:, tsl], op=ALU.mult)

        # ---- phase 3: main loop over (f-chunk, t-tile), software-pipelined ----
        # stage A(i): base + delta matmuls, bias-add; stage B(i): relu-combine.
        # B(i) is emitted after A(i+1) so the PE never waits on the DVE.
        with (
            tc.tile_pool(name="wi_sb", bufs=3) as wi_pool,
            tc.tile_pool(name="lb_sb", bufs=3) as lb_pool,
            tc.tile_pool(name="base_ps", bufs=2, space="PSUM") as base_pool,
            tc.tile_pool(name="bank_ps", bufs=6, space="PSUM") as bank_pool,
            tc.tile_pool(name="bs_sb", bufs=3) as bs_pool,
            tc.tile_pool(name="s_sb", bufs=6) as s_pool,
            tc.tile_pool(name="r_sb2", bufs=3) as rr_pool,
        ):
            tiles = [(fc, tt) for fc in range(FCH) for tt in range(NTT)]
            pend = {}
            wi_cur = lb_cur = None
            for i in range(len(tiles) + 1):
                if i < len(tiles):
                    fc, tt = tiles[i]
                    tsl = slice(tt * TT, (tt + 1) * TT)
                    if tt == 0:
                        wi_cur = wi_pool.tile([128, DCH, 128], BF16, tag="wi",
                                              name=f"wi{fc}")
                        nc.sync.dma_start(out=wi_cur, in_=wiB[fc])
                        lb_cur = lb_pool.tile([128, 128], BF16, tag="lb",
                                              name=f"lb{fc}")
                        nc.sync.dma_start(out=lb_cur, in_=lBc[fc])
                    banks = []
                    for tA_m in (tA1_sb, tA2_sb):
                        bank = bank_pool.tile([128, TT], F32, tag="bank")
                        nc.tensor.matmul(
                            bank, lhsT=lb_cur, rhs=tA_m[:, tsl],
                            start=True, stop=True,
                        )
                        banks.append(bank)
                    bps = base_pool.tile([128, TT], F32, tag="base")
                    for ci in range(DCH):
                        nc.tensor.matmul(
                            bps,
                            lhsT=wi_cur[:, ci, :],
                            rhs=hsB_sb[:, ci, tsl],
                            start=(ci == 0), stop=(ci == DCH - 1),
                        )
                    bs = bs_pool.tile([128, TT], BF16, tag="bs")
                    nc.vector.tensor_scalar(
                        out=bs, in0=bps, scalar1=bi_sb[:, fc:fc + 1],
                        scalar2=None, op0=ALU.add)
                    pend[i] = (fc, tt, tsl, banks, bs)
                if i - 1 in pend:
                    fc0, tt0, tsl0, banks0, bs0 = pend.pop(i - 1)
                    # s_k = delta_k + base ; acc = relu(s1) + relu(s2)
                    s1 = s_pool.tile([128, TT], BF16, tag="s")
                    nc.vector.scalar_tensor_tensor(
                        out=s1, in0=banks0[0], scalar=0.0, in1=bs0,
                        op0=ALU.add, op1=ALU.add)
                    s2 = s_pool.tile([128, TT], BF16, tag="s")
                    nc.vector.scalar_tensor_tensor(
                        out=s2, in0=banks0[1], scalar=0.0, in1=bs0,
                        op0=ALU.add, op1=ALU.add)
                    r2 = rr_pool.tile([128, TT], BF16, tag="r")
                    nc.scalar.activation(r2, s2, AF.Relu)
                    acc = rr_pool.tile([128, TT], BF16, tag="acc")
                    nc.vector.scalar_tensor_tensor(
                        out=acc, in0=s1, scalar=0.0, in1=r2,
                        op0=ALU.max, op1=ALU.add)
                    nc.vector.tensor_tensor(
                        out=H_sb[fc0][:, tsl0], in0=acc, in1=V_b[:, tsl0],
                        op=ALU.mult)

        # ---- phase 4: wo matmul (+ rank-1 2*v*bo term) ----
        with (
            tc.tile_pool(name="wo_sb", bufs=2) as wo_pool,
            tc.tile_pool(name="wo_ps", bufs=1, space="PSUM") as wo_ps,
            tc.tile_pool(name="o_sb", bufs=3) as o_pool,
        ):
            for tt in range(NTT):
                tsl = slice(tt * TT, (tt + 1) * TT)
                ops = [wo_ps.tile([128, TT], F32, tag=f"o{dc}", name=f"o{tt}_{dc}")
                       for dc in range(8)]
                for fc in range(FCH):
                    wo_t = wo_pool.tile([128, DM], BF16, tag="wo")
                    nc.sync.dma_start(out=wo_t, in_=woB[fc])
                    for dc in range(8):
                        nc.tensor.matmul(
                            ops[dc],
                            lhsT=wo_t[:, dc * 128:(dc + 1) * 128],
                            rhs=H_sb[fc][:, tsl],
                            start=(fc == 0), stop=False,
                        )
                for dc in range(8):
                    nc.tensor.matmul(
                        ops[dc],
                        lhsT=bo2_sb[0:1, dc * 128:(dc + 1) * 128],
                        rhs=vTb_sb[0:1, tsl],
                        start=False, stop=True,
                    )
                    o_t = o_pool.tile([128, TT], F32, tag="ot")
                    nc.scalar.copy(out=o_t, in_=ops[dc])
                    nc.sync.dma_start(
                        out=outT[dc * 128:(dc + 1) * 128, tsl], in_=o_t
                    )

    nc.compile()
    return nc


def prep_inputs(hidden_states, wi, bi, wo, bo, lora_A, lora_B, gate_w, gate_b):
    """Host-side layout prep; returns per-core input maps."""
    import ml_dtypes
    bf = ml_dtypes.bfloat16
    f32 = np.float32
    hs = np.asarray(hidden_states, f32).reshape(TOK, DM)
    wi = np.asarray(wi, f32); bi = np.asarray(bi, f32)
    wo = np.asarray(wo, f32); bo = np.asarray(bo, f32)
    lora_A = np.asarray(lora_A, f32); lora_B = np.asarray(lora_B, f32)
    gate_w = np.asarray(gate_w, f32); gate_b = np.asarray(gate_b, f32)

    # wi^T in (fc, d, ci, f) bf16 chunks
    wiB = np.ascontiguousarray(
        wi.T.reshape(DCH, 128, FCH, 128).transpose(2, 1, 0, 3)).astype(bf)
    biC = np.ascontiguousarray(bi.reshape(FCH, 128).T)

    # lora-A concatenated: columns 16e+r = lora_A[e,r,:]
    lA_cat = np.concatenate([lora_A[e].T for e in range(E)], axis=1)  # [DM,128]
    lAc = np.ascontiguousarray(lA_cat.reshape(DCH, 128, 128).transpose(1, 0, 2)
                               ).astype(bf)
    # lora-B concatenated: rows 16e+r = lora_B[e,:,r]
    lB_cat = np.concatenate([lora_B[e].T for e in range(E)], axis=0)  # [128,FF]
    lBc = np.ascontiguousarray(
        lB_cat.reshape(128, FCH, 128).transpose(1, 0, 2)).astype(bf)

    KD = NKC * 128
    gwA = np.zeros((KD, E), f32)
    gwA[:DM] = gate_w.T
    gwA[DM] = gate_b
    gw9 = np.ascontiguousarray(gwA.reshape(NKC, 128, E).transpose(1, 0, 2))

    woB = np.ascontiguousarray(wo.T.reshape(FCH, 128, DM)).astype(bf)
    bo2 = (2.0 * bo).astype(bf).reshape(1, DM)

    shared = dict(wiB=wiB, biC=biC, lAc=lAc, lBc=lBc, gw9=gw9, woB=woB, bo2=bo2)
    in_maps = []
    for c in range(NCORES):
        hsc = hs[c * T:(c + 1) * T]
        hsA = np.zeros((KD, T), f32)
        hsA[:DM] = hsc.T
        hsA[DM] = 1.0
        hsR = np.ascontiguousarray(hsA.reshape(NKC, 128, T).transpose(1, 0, 2))
        hsB = np.ascontiguousarray(
            hsc.T.reshape(DCH, 128, T).transpose(1, 0, 2)).astype(bf)
        in_maps.append(dict(hsR=hsR, hsB=hsB, **shared))
    return in_maps


def run(in_maps, **kwargs):
    nc = build_bass()
    return nc, run_bass_kernel_spmd(nc, in_maps, list(range(NCORES)), **kwargs)


def kernel(hidden_states, wi, bi, wo, bo, lora_A, lora_B, gate_w, gate_b):
    in_maps = prep_inputs(hidden_states, wi, bi, wo, bo, lora_A, lora_B,
                          gate_w, gate_b)
    _, res = run(in_maps)
    out = np.stack([res.results[c]["outT"].T for c in range(NCORES)])
    return out.reshape(B, S, DM).astype(np.float32)


# revision 15
# speedup vs baseline: 1.1314x; 1.1314x over previous
"""Trainium2 Bass kernel for the MoE block (nn_MoEBlock_5592047420171).

Strategy: data-parallel over tokens across 8 NeuronCores (1024 tokens/core,
all weights replicated; no collectives).  Per core, layout A (d_ff on
partitions, tokens on the free dim):

  out[t,:] = v_t * (relu(base_t + delta_{e1(t),t}) + relu(base_t + delta_{e2(t),t}))
             @ wo^T + 2 v_t * bo
  base = hs wi^T + bi,  delta_e = (hs lA_e^T) lB_e^T,  v = top2 softmax mass

Key structure:
  * router computes P=exp(logits) in real fp32 (so top-2 selection matches the
    fp32 reference); the top-8 DVE sort gives max/second-max; one-hot masks of
    the first/second choice expert are built per token.
  * the 8 per-expert rank-16 lora paths collapse to TWO dense K=128 matmuls:
    tA (all experts' lora-A outputs, 8x16 rows) is masked per token by the
    first/second-choice one-hot (16-row groups), then multiplied by the
    concatenated lora-B.  No per-expert loop on the hot path.
  * base is computed once per tile and added into each choice's PSUM bank
    with an identity matmul; bi rides the PSUM->SBUF copy as an ACT bias.
  * val_sum v is pulled out of the expert sum (one broadcast multiply per
    tile); 2*v*bo rides the wo matmul as a K=1 rank-1 term.
  * all big matmuls are bf16 (fp32r measured at half the bf16 rate).
"""

import numpy as np
from contextlib import ExitStack

import concourse.bass as bass
import concourse.tile as tile
from concourse import bacc, mybir
from concourse.bass_utils import run_bass_kernel_spmd
from concourse.masks import make_identity

B, S, DM, FF, E, RK = 4, 2048, 1024, 4096, 8, 16
NCORES = 8
TOK = B * S            # 8192 tokens
T = TOK // NCORES      # 1024 tokens per core
TCH = T // 128         # 8 token chunks of 128
FCH = FF // 128        # 32 d_ff chunks of 128
TT = 512               # token tile width (free dim of big matmuls)
NTT = T // TT          # 2 token tiles
NKC = 9                # router contraction chunks: 8 x 128 d_model + bias
DCH = 8                # d_model chunks for the bf16 matmuls

F32 = mybir.dt.float32
BF16 = mybir.dt.bfloat16
AX = mybir.AxisListType
ALU = mybir.AluOpType
AF = mybir.ActivationFunctionType


def build_bass():
    nc = bacc.Bacc("TRN2", target_bir_lowering=False)

    hsR = nc.declare_dram_parameter("hsR", [128, NKC, T], F32, isOutput=False)
    hsB = nc.declare_dram_parameter("hsB", [128, DCH, T], BF16, isOutput=False)
    wiB = nc.declare_dram_parameter("wiB", [FCH, 128, DCH, 128], BF16, isOutput=False)
    biC = nc.declare_dram_parameter("biC", [128, FCH], F32, isOutput=False)
    lAc = nc.declare_dram_parameter("lAc", [128, DCH, 128], BF16, isOutput=False)
    lBc = nc.declare_dram_parameter("lBc", [FCH, 128, 128], BF16, isOutput=False)
    gw9 = nc.declare_dram_parameter("gw9", [128, NKC, 8], F32, isOutput=False)
    woB = nc.declare_dram_parameter("woB", [FCH, 128, DM], BF16, isOutput=False)
    bo2 = nc.declare_dram_parameter("bo2", [1, DM], BF16, isOutput=False)
    outT = nc.declare_dram_parameter("outT", [DM, T], F32, isOutput=True)

    hsR, hsB, wiB, biC, lAc, lBc, gw9, woB, bo2, outT = (
        h.ap() for h in (hsR, hsB, wiB, biC, lAc, lBc, gw9, woB, bo2, outT))

    with tile.TileContext(nc) as tc, ExitStack() as ctx:
        persist = ctx.enter_context(tc.tile_pool(name="persist", bufs=1))
        dram = ctx.enter_context(tc.tile_pool(name="dram", bufs=1, space="DRAM"))

        # ---- resident tensors ----
        hsR_sb = persist.tile([128, NKC, T], F32, tag="hsR")
        nc.sync.dma_start(out=hsR_sb, in_=hsR)
        hsB_sb = persist.tile([128, DCH, T], BF16, tag="hsB")
        nc.sync.dma_start(out=hsB_sb, in_=hsB)
        gw_sb = persist.tile([128, NKC, 8], F32, tag="gw")
        nc.sync.dma_start(out=gw_sb, in_=gw9)
        lA_sb = persist.tile([128, DCH, 128], BF16, tag="lA")
        nc.sync.dma_start(out=lA_sb, in_=lAc)
        bi_sb = persist.tile([128, FCH], F32, tag="bi")
        nc.sync.dma_start(out=bi_sb, in_=biC)
        bo2_sb = persist.tile([1, DM], BF16, tag="bo2")
        nc.sync.dma_start(out=bo2_sb, in_=bo2)
        ident = persist.tile([128, 128], F32, tag="ident")
        make_identity(nc, ident)
        identb = persist.tile([128, 128], BF16, tag="identb")
        nc.scalar.copy(out=identb, in_=ident)

        ohT_sb = persist.tile([16, T], BF16, tag="ohT")   # oh1 rows 0-7, oh2 8-15
        vT_sb = persist.tile([1, T], F32, tag="vT")       # val_sum row (f32)
        vTb_sb = persist.tile([1, T], BF16, tag="vTb")    # val_sum row (bf16)
        V_b = persist.tile([128, T], BF16, tag="Vb")      # val_sum bcast
        M1_sb = persist.tile([128, T], BF16, tag="M1")    # first-choice mask
        M2_sb = persist.tile([128, T], BF16, tag="M2")    # second-choice mask
        tA1_sb = persist.tile([128, T], BF16, tag="tA1")  # masked lora-A (1st)
        tA2_sb = persist.tile([128, T], BF16, tag="tA2")  # masked lora-A (2nd)
        H_sb = [persist.tile([128, T], BF16, tag=f"H{fc}", name=f"H{fc}")
                for fc in range(FCH)]
        vrow = dram.tile([1, T], BF16, tag="vrow")
        ohd = dram.tile([16, T], BF16, tag="ohd")

        # ---- phase 1: router ----
        with (
            tc.tile_pool(name="r_ps", bufs=2, space="PSUM") as r_ps,
            tc.tile_pool(name="tr_ps", bufs=2, space="PSUM") as tr_ps,
            tc.tile_pool(name="r_sb", bufs=3) as r_sb,
        ):
            for tch in range(TCH):
                tsl = slice(tch * 128, (tch + 1) * 128)
                lg = r_ps.tile([128, 8], F32, tag="lg")
                for ci in range(NKC):
                    nc.tensor.matmul(
                        lg,
                        lhsT=hsR_sb[:, ci, tsl],
                        rhs=gw_sb[:, ci, :],
                        start=(ci == 0), stop=(ci == NKC - 1),
                    )
                P = r_sb.tile([128, 8], F32, tag="P")
                nc.scalar.activation(P, lg, AF.Exp)
                top8 = r_sb.tile([128, 8], F32, tag="top8")
                nc.vector.max(out=top8, in_=P)
                sP = r_sb.tile([128, 1], F32, tag="sP")
                nc.vector.tensor_reduce(out=sP, in_=P, axis=AX.X, op=ALU.add)
                rv = r_sb.tile([128, 1], F32, tag="rv")
                nc.vector.reciprocal(rv, sP)
                # oh1 = (P == max); oh2 = (P >= m2) - oh1 ; v = (m1+m2)*rv
                ohb = r_sb.tile([128, 16], F32, tag="ohb")
                nc.vector.tensor_scalar(
                    out=ohb[:, 0:8], in0=P, scalar1=top8[:, 0:1], scalar2=None,
                    op0=ALU.is_equal,
                )
                nc.vector.scalar_tensor_tensor(
                    out=ohb[:, 8:16], in0=P, scalar=top8[:, 1:2],
                    in1=ohb[:, 0:8], op0=ALU.is_ge, op1=ALU.subtract,
                )
                vc = r_sb.tile([128, 1], F32, tag="vc")
                nc.vector.scalar_tensor_tensor(
                    out=vc, in0=top8[:, 0:1], scalar=top8[:, 1:2],
                    in1=rv, op0=ALU.add, op1=ALU.mult,
                )
                trp = tr_ps.tile([16, 128], F32, tag="trp")
                nc.tensor.transpose(trp, ohb, ident)
                trpv = tr_ps.tile([1, 128], F32, tag="trpv")
                nc.tensor.transpose(trpv, vc, ident)
                nc.scalar.copy(out=ohT_sb[:, tsl], in_=trp)
                nc.scalar.copy(out=vT_sb[:, tsl], in_=trpv[0:1, :])

        nc.scalar.copy(out=vTb_sb, in_=vT_sb)
        nc.sync.dma_start(out=vrow, in_=vTb_sb)
        nc.sync.dma_start(out=V_b, in_=vrow.to_broadcast([128, T]))
        nc.sync.dma_start(out=ohd, in_=ohT_sb)
        # expand one-hot rows to 16-row groups: M[16e+k, t] = oh[e, t]
        m1_src = bass.AP(tensor=ohd.tensor, offset=ohd.offset,
                         ap=[[T, 8], [0, 16], [1, T]])
        nc.sync.dma_start(out=M1_sb, in_=m1_src)
        m2_src = bass.AP(tensor=ohd.tensor, offset=ohd.offset + 8 * T,
                         ap=[[T, 8], [0, 16], [1, T]])
        nc.sync.dma_start(out=M2_sb, in_=m2_src)

        # ---- phase 2: lora-A projections + per-token choice masking ----
        with (
            tc.tile_pool(name="tA_ps", bufs=2, space="PSUM") as tA_ps,
            tc.tile_pool(name="tA_tmp", bufs=2) as tA_tmp,
        ):
            for tt in range(NTT):
                tsl = slice(tt * TT, (tt + 1) * TT)
                pta = tA_ps.tile([128, TT], F32, tag="pta")
                for ci in range(DCH):
                    nc.tensor.matmul(
                        pta,
                        lhsT=lA_sb[:, ci, :],
                        rhs=hsB_sb[:, ci, tsl],
                        start=(ci == 0), stop=(ci == DCH - 1),
                    )
                tAf = tA_tmp.tile([128, TT], BF16, tag="tAf")
                nc.scalar.copy(out=tAf, in_=pta)
                nc.vector.tensor_tensor(
                    out=tA1_sb[:, tsl], in0=tAf, in1=M1_sb[:, tsl], op=ALU.mult)
                nc.vector.tensor_tensor(
                    out=tA2_sb[:, tsl], in0=tAf, in1=M2_sb[:, tsl], op=ALU.mult)

        # ---- phase 3: main loop over (f-chunk, t-tile), software-pipelined ----
        # stage A(i): base + delta matmuls, bias-add; stage B(i): relu-combine.
        # B(i) is emitted after A(i+1) so the PE never waits on the DVE.
        with (
            tc.tile_pool(name="wi_sb", bufs=3) as wi_pool,
            tc.tile_pool(name="lb_sb", bufs=3) as lb_pool,
            tc.tile_pool(name="base_ps", bufs=2, space="PSUM") as base_pool,
            tc.tile_pool(name="bank_ps", bufs=6, space="PSUM") as bank_pool,
            tc.tile_pool(name="bs_sb", bufs=3) as bs_pool,
            tc.tile_pool(name="s_sb", bufs=6) as s_pool,
            tc.tile_pool(name="r_sb2", bufs=3) as rr_pool,
        ):
            tiles = [(fc, tt) for fc in range(FCH) for tt in range(NTT)]
            pend = {}
            wi_cur = lb_cur = None
            for i in range(len(tiles) + 1):
                if i < len(tiles):
                    fc, tt = tiles[i]
                    tsl = slice(tt * TT, (tt + 1) * TT)
                    if tt == 0:
                        wi_cur = wi_pool.tile([128, DCH, 128], BF16, tag="wi",
                                              name=f"wi{fc}")
                        nc.sync.dma_start(out=wi_cur, in_=wiB[fc])
                        lb_cur = lb_pool.tile([128, 128], BF16, tag="lb",
                                              name=f"lb{fc}")
                        nc.sync.dma_start(out=lb_cur, in_=lBc[fc])
                    banks = []
                    for tA_m in (tA1_sb, tA2_sb):
                        bank = bank_pool.tile([128, TT], F32, tag="bank")
                        nc.tensor.matmul(
                            bank, lhsT=lb_cur, rhs=tA_m[:, tsl],
                            start=True, stop=(tA_m is tA2_sb),
                        )
                        banks.append(bank)
                    bps = base_pool.tile([128, TT], F32, tag="base")
                    for ci in range(DCH):
                        nc.tensor.matmul(
                            bps,
                            lhsT=wi_cur[:, ci, :],
                            rhs=hsB_sb[:, ci, tsl],
                            start=(ci == 0), stop=(ci == DCH - 1),
                        )
                    bs = bs_pool.tile([128, TT], BF16, tag="bs")
                    nc.vector.tensor_scalar(
                        out=bs, in0=bps, scalar1=bi_sb[:, fc:fc + 1],
                        scalar2=None, op0=ALU.add)
                    # choice 1: base-add via identity matmul on the PE
                    nc.tensor.matmul(
                        banks[0], lhsT=identb, rhs=bs,
                        start=False, stop=True,
                    )
                    pend[i] = (fc, tt, tsl, banks, bs)
                if i - 1 in pend:
                    fc0, tt0, tsl0, banks0, bs0 = pend.pop(i - 1)
                    # choice 1: relu on ACT; choice 2: base-add + relu on DVE
                    r1 = rr_pool.tile([128, TT], BF16, tag="r")
                    nc.scalar.activation(r1, banks0[0], AF.Relu)
                    s2 = s_pool.tile([128, TT], BF16, tag="s")
                    nc.vector.scalar_tensor_tensor(
                        out=s2, in0=banks0[1], scalar=0.0, in1=bs0,
                        op0=ALU.add, op1=ALU.add)
                    acc = rr_pool.tile([128, TT], BF16, tag="acc")
                    nc.vector.scalar_tensor_tensor(
                        out=acc, in0=s2, scalar=0.0, in1=r1,
                        op0=ALU.max, op1=ALU.add)
                    nc.vector.tensor_tensor(
                        out=H_sb[fc0][:, tsl0], in0=acc, in1=V_b[:, tsl0],
                        op=ALU.mult)

        # ---- phase 4: wo matmul (+ rank-1 2*v*bo term) ----
        with (
            tc.tile_pool(name="wo_sb", bufs=2) as wo_pool,
            tc.tile_pool(name="wo_ps", bufs=1, space="PSUM") as wo_ps,
            tc.tile_pool(name="o_sb", bufs=3) as o_pool,
        ):
            for tt in range(NTT):
                tsl = slice(tt * TT, (tt + 1) * TT)
                ops = [wo_ps.tile([128, TT], F32, tag=f"o{dc}", name=f"o{tt}_{dc}")
                       for dc in range(8)]
                for fc in range(FCH):
                    wo_t = wo_pool.tile([128, DM], BF16, tag="wo")
                    nc.sync.dma_start(out=wo_t, in_=woB[fc])
                    for dc in range(8):
                        nc.tensor.matmul(
                            ops[dc],
                            lhsT=wo_t[:, dc * 128:(dc + 1) * 128],
                            rhs=H_sb[fc][:, tsl],
                            start=(fc == 0), stop=False,
                        )
                for dc in range(8):
                    nc.tensor.matmul(
                        ops[dc],
                        lhsT=bo2_sb[0:1, dc * 128:(dc + 1) * 128],
                        rhs=vTb_sb[0:1, tsl],
                        start=False, stop=True,
                    )
                    o_t = o_pool.tile([128, TT], F32, tag="ot")
                    nc.scalar.copy(out=o_t, in_=ops[dc])
                    nc.sync.dma_start(
                        out=outT[dc * 128:(dc + 1) * 128, tsl], in_=o_t
                    )

    nc.compile()
    return nc


def prep_inputs(hidden_states, wi, bi, wo, bo, lora_A, lora_B, gate_w, gate_b):
    """Host-side layout prep; returns per-core input maps."""
    import ml_dtypes
    bf = ml_dtypes.bfloat16
    f32 = np.float32
    hs = np.asarray(hidden_states, f32).reshape(TOK, DM)
    wi = np.asarray(wi, f32); bi = np.asarray(bi, f32)
    wo = np.asarray(wo, f32); bo = np.asarray(bo, f32)
    lora_A = np.asarray(lora_A, f32); lora_B = np.asarray(lora_B, f32)
    gate_w = np.asarray(gate_w, f32); gate_b = np.asarray(gate_b, f32)

    # wi^T in (fc, d, ci, f) bf16 chunks
    wiB = np.ascontiguousarray(
        wi.T.reshape(DCH, 128, FCH, 128).transpose(2, 1, 0, 3)).astype(bf)
    biC = np.ascontiguousarray(bi.reshape(FCH, 128).T)

    # lora-A concatenated: columns 16e+r = lora_A[e,r,:]
    lA_cat = np.concatenate([lora_A[e].T for e in range(E)], axis=1)  # [DM,128]
    lAc = np.ascontiguousarray(lA_cat.reshape(DCH, 128, 128).transpose(1, 0, 2)
                               ).astype(bf)
    # lora-B concatenated: rows 16e+r = lora_B[e,:,r]
    lB_cat = np.concatenate([lora_B[e].T for e in range(E)], axis=0)  # [128,FF]
    lBc = np.ascontiguousarray(
        lB_cat.reshape(128, FCH, 128).transpose(1, 0, 2)).astype(bf)

    KD = NKC * 128
    gwA = np.zeros((KD, E), f32)
    gwA[:DM] = gate_w.T
    gwA[DM] = gate_b
    gw9 = np.ascontiguousarray(gwA.reshape(NKC, 128, E).transpose(1, 0, 2))

    woB = np.ascontiguousarray(wo.T.reshape(FCH, 128, DM)).astype(bf)
    bo2 = (2.0 * bo).astype(bf).reshape(1, DM)

    shared = dict(wiB=wiB, biC=biC, lAc=lAc, lBc=lBc, gw9=gw9, woB=woB, bo2=bo2)
    in_maps = []
    for c in range(NCORES):
        hsc = hs[c * T:(c + 1) * T]
        hsA = np.zeros((KD, T), f32)
        hsA[:DM] = hsc.T
        hsA[DM] = 1.0
        hsR = np.ascontiguousarray(hsA.reshape(NKC, 128, T).transpose(1, 0, 2))
        hsB = np.ascontiguousarray(
            hsc.T.reshape(DCH, 128, T).transpose(1, 0, 2)).astype(bf)
        in_maps.append(dict(hsR=hsR, hsB=hsB, **shared))
    return in_maps


def run(in_maps, **kwargs):
    nc = build_bass()
    return nc, run_bass_kernel_spmd(nc, in_maps, list(range(NCORES)), **kwargs)


def kernel(hidden_states, wi, bi, wo, bo, lora_A, lora_B, gate_w, gate_b):
    in_maps = prep_inputs(hidden_states, wi, bi, wo, bo, lora_A, lora_B,
                          gate_w, gate_b)
    _, res = run(in_maps)
    out = np.stack([res.results[c]["outT"].T for c in range(NCORES)])
    return out.reshape(B, S, DM).astype(np.float32)


# revision 17
# speedup vs baseline: 1.2919x; 1.1418x over previous
"""Trainium2 Bass kernel for the MoE block (nn_MoEBlock_5592047420171).

Strategy: data-parallel over tokens across 8 NeuronCores (1024 tokens/core,
all weights replicated; no collectives).  Per core, layout A (d_ff on
partitions, tokens on the free dim):

  out[t,:] = v_t * (relu(base_t + delta_{e1(t),t}) + relu(base_t + delta_{e2(t),t}))
             @ wo^T + 2 v_t * bo
  base = hs wi^T + bi,  delta_e = (hs lA_e^T) lB_e^T,  v = top2 softmax mass

Key structure:
  * router computes P=exp(logits) in real fp32 (so top-2 selection matches the
    fp32 reference); the top-8 DVE sort gives max/second-max; one-hot masks of
    the first/second choice expert are built per token.
  * the 8 per-expert rank-16 lora paths collapse to TWO dense K=128 matmuls:
    tA (all experts' lora-A outputs, 8x16 rows) is masked per token by the
    first/second-choice one-hot (16-row groups), then multiplied by the
    concatenated lora-B.  No per-expert loop on the hot path.
  * base is computed once per tile and added into each choice's PSUM bank
    with an identity matmul; bi rides the PSUM->SBUF copy as an ACT bias.
  * val_sum v is pulled out of the expert sum (one broadcast multiply per
    tile); 2*v*bo rides the wo matmul as a K=1 rank-1 term.
  * all big matmuls are bf16 (fp32r measured at half the bf16 rate).
"""

import numpy as np
from contextlib import ExitStack

import concourse.bass as bass
import concourse.tile as tile
from concourse import bacc, mybir
from concourse.bass_utils import run_bass_kernel_spmd
from concourse.masks import make_identity

B, S, DM, FF, E, RK = 4, 2048, 1024, 4096, 8, 16
NCORES = 8
TOK = B * S            # 8192 tokens
T = TOK // NCORES      # 1024 tokens per core
TCH = T // 128         # 8 token chunks of 128
FCH = FF // 128        # 32 d_ff chunks of 128
TT = 512               # token tile width (free dim of big matmuls)
NTT = T // TT          # 2 token tiles
NKC = 9                # router contraction chunks: 8 x 128 d_model + bias
DCH = 8                # d_model chunks for the bf16 matmuls

F32 = mybir.dt.float32
BF16 = mybir.dt.bfloat16
AX = mybir.AxisListType
ALU = mybir.AluOpType
AF = mybir.ActivationFunctionType


def build_bass():
    nc = bacc.Bacc("TRN2", target_bir_lowering=False)

    hsR = nc.declare_dram_parameter("hsR", [128, NKC, T], F32, isOutput=False)
    hsB = nc.declare_dram_parameter("hsB", [128, DCH, T], BF16, isOutput=False)
    wiB = nc.declare_dram_parameter("wiB", [FCH, 128, DCH, 128], BF16, isOutput=False)
    biC = nc.declare_dram_parameter("biC", [128, FCH], F32, isOutput=False)
    lAc = nc.declare_dram_parameter("lAc", [128, DCH, 128], BF16, isOutput=False)
    lBc = nc.declare_dram_parameter("lBc", [FCH, 128, 128], BF16, isOutput=False)
    gw9 = nc.declare_dram_parameter("gw9", [128, NKC, 8], F32, isOutput=False)
    woB = nc.declare_dram_parameter("woB", [FCH, 128, DM], BF16, isOutput=False)
    bo2 = nc.declare_dram_parameter("bo2", [1, DM], BF16, isOutput=False)
    outT = nc.declare_dram_parameter("outT", [DM, T], F32, isOutput=True)

    hsR, hsB, wiB, biC, lAc, lBc, gw9, woB, bo2, outT = (
        h.ap() for h in (hsR, hsB, wiB, biC, lAc, lBc, gw9, woB, bo2, outT))

    with tile.TileContext(nc) as tc, ExitStack() as ctx:
        persist = ctx.enter_context(tc.tile_pool(name="persist", bufs=1))
        dram = ctx.enter_context(tc.tile_pool(name="dram", bufs=1, space="DRAM"))

        # ---- resident tensors ----
        hsR_cm = tc.tile_pool(name="hsR_pool", bufs=1)
        hsR_pool = hsR_cm.__enter__()
        hsR_sb = hsR_pool.tile([128, NKC, T], F32, tag="hsR")
        nc.sync.dma_start(out=hsR_sb, in_=hsR)
        hsB_sb = persist.tile([128, DCH, T], BF16, tag="hsB")
        nc.sync.dma_start(out=hsB_sb, in_=hsB)
        gw_sb = persist.tile([128, NKC, 8], F32, tag="gw")
        nc.sync.dma_start(out=gw_sb, in_=gw9)
        lA_sb = persist.tile([128, DCH, 128], BF16, tag="lA")
        nc.sync.dma_start(out=lA_sb, in_=lAc)
        bi_sb = persist.tile([128, FCH], F32, tag="bi")
        nc.sync.dma_start(out=bi_sb, in_=biC)
        bo2_sb = persist.tile([1, DM], BF16, tag="bo2")
        nc.sync.dma_start(out=bo2_sb, in_=bo2)
        ident = persist.tile([128, 128], F32, tag="ident")
        make_identity(nc, ident)
        identb = persist.tile([128, 128], BF16, tag="identb")
        nc.scalar.copy(out=identb, in_=ident)

        ohT_sb = persist.tile([16, T], BF16, tag="ohT")   # oh1 rows 0-7, oh2 8-15
        vT_sb = persist.tile([1, T], F32, tag="vT")       # val_sum row (f32)
        vTb_sb = persist.tile([1, T], BF16, tag="vTb")    # val_sum row (bf16)
        V_b = persist.tile([128, T], BF16, tag="Vb")      # val_sum bcast
        M1_sb = persist.tile([128, T], BF16, tag="M1")    # first-choice mask
        M2_sb = persist.tile([128, T], BF16, tag="M2")    # second-choice mask
        tA1_sb = persist.tile([128, T], BF16, tag="tA1")  # masked lora-A (1st)
        tA2_sb = persist.tile([128, T], BF16, tag="tA2")  # masked lora-A (2nd)
        H_sb = [persist.tile([128, T], BF16, tag=f"H{fc}", name=f"H{fc}")
                for fc in range(FCH)]
        vrow = dram.tile([1, T], BF16, tag="vrow")
        ohd = dram.tile([16, T], BF16, tag="ohd")

        # ---- phase 1: router ----
        with (
            tc.tile_pool(name="r_ps", bufs=2, space="PSUM") as r_ps,
            tc.tile_pool(name="tr_ps", bufs=2, space="PSUM") as tr_ps,
            tc.tile_pool(name="r_sb", bufs=3) as r_sb,
        ):
            for tch in range(TCH):
                tsl = slice(tch * 128, (tch + 1) * 128)
                lg = r_ps.tile([128, 8], F32, tag="lg")
                for ci in range(NKC):
                    nc.tensor.matmul(
                        lg,
                        lhsT=hsR_sb[:, ci, tsl],
                        rhs=gw_sb[:, ci, :],
                        start=(ci == 0), stop=(ci == NKC - 1),
                    )
                P = r_sb.tile([128, 8], F32, tag="P")
                nc.scalar.activation(P, lg, AF.Exp)
                top8 = r_sb.tile([128, 8], F32, tag="top8")
                nc.vector.max(out=top8, in_=P)
                sP = r_sb.tile([128, 1], F32, tag="sP")
                nc.vector.tensor_reduce(out=sP, in_=P, axis=AX.X, op=ALU.add)
                rv = r_sb.tile([128, 1], F32, tag="rv")
                nc.vector.reciprocal(rv, sP)
                # oh1 = (P == max); oh2 = (P >= m2) - oh1 ; v = (m1+m2)*rv
                ohb = r_sb.tile([128, 16], F32, tag="ohb")
                nc.vector.tensor_scalar(
                    out=ohb[:, 0:8], in0=P, scalar1=top8[:, 0:1], scalar2=None,
                    op0=ALU.is_equal,
                )
                nc.vector.scalar_tensor_tensor(
                    out=ohb[:, 8:16], in0=P, scalar=top8[:, 1:2],
                    in1=ohb[:, 0:8], op0=ALU.is_ge, op1=ALU.subtract,
                )
                vc = r_sb.tile([128, 1], F32, tag="vc")
                nc.vector.scalar_tensor_tensor(
                    out=vc, in0=top8[:, 0:1], scalar=top8[:, 1:2],
                    in1=rv, op0=ALU.add, op1=ALU.mult,
                )
                trp = tr_ps.tile([16, 128], F32, tag="trp")
                nc.tensor.transpose(trp, ohb, ident)
                trpv = tr_ps.tile([1, 128], F32, tag="trpv")
                nc.tensor.transpose(trpv, vc, ident)
                nc.scalar.copy(out=ohT_sb[:, tsl], in_=trp)
                nc.scalar.copy(out=vT_sb[:, tsl], in_=trpv[0:1, :])

        hsR_cm.__exit__(None, None, None)
        wo_all = persist.tile([128, FCH, DM], BF16, tag="woall")
        nc.sync.dma_start(out=wo_all, in_=woB.transpose([1, 0, 2]))

        nc.scalar.copy(out=vTb_sb, in_=vT_sb)
        nc.sync.dma_start(out=vrow, in_=vTb_sb)
        nc.sync.dma_start(out=V_b, in_=vrow.to_broadcast([128, T]))
        nc.sync.dma_start(out=ohd, in_=ohT_sb)
        # expand one-hot rows to 16-row groups: M[16e+k, t] = oh[e, t]
        m1_src = bass.AP(tensor=ohd.tensor, offset=ohd.offset,
                         ap=[[T, 8], [0, 16], [1, T]])
        nc.sync.dma_start(out=M1_sb, in_=m1_src)
        m2_src = bass.AP(tensor=ohd.tensor, offset=ohd.offset + 8 * T,
                         ap=[[T, 8], [0, 16], [1, T]])
        nc.sync.dma_start(out=M2_sb, in_=m2_src)

        # ---- phase 2: lora-A projections + per-token choice masking ----
        with (
            tc.tile_pool(name="tA_ps", bufs=2, space="PSUM") as tA_ps,
            tc.tile_pool(name="tA_tmp", bufs=2) as tA_tmp,
        ):
            for tt in range(NTT):
                tsl = slice(tt * TT, (tt + 1) * TT)
                pta = tA_ps.tile([128, TT], F32, tag="pta")
                for ci in range(DCH):
                    nc.tensor.matmul(
                        pta,
                        lhsT=lA_sb[:, ci, :],
                        rhs=hsB_sb[:, ci, tsl],
                        start=(ci == 0), stop=(ci == DCH - 1),
                    )
                tAf = tA_tmp.tile([128, TT], BF16, tag="tAf")
                nc.scalar.copy(out=tAf, in_=pta)
                nc.vector.tensor_tensor(
                    out=tA1_sb[:, tsl], in0=tAf, in1=M1_sb[:, tsl], op=ALU.mult)
                nc.vector.tensor_tensor(
                    out=tA2_sb[:, tsl], in0=tAf, in1=M2_sb[:, tsl], op=ALU.mult)

        # ---- phase 3: main loop over (f-chunk, t-tile), software-pipelined ----
        # stage A(i): base + delta matmuls, bias-add; stage B(i): relu-combine.
        # B(i) is emitted after A(i+1) so the PE never waits on the DVE.
        with (
            tc.tile_pool(name="wi_sb", bufs=6) as wi_pool,
            tc.tile_pool(name="lb_sb", bufs=6) as lb_pool,
            tc.tile_pool(name="base_ps", bufs=2, space="PSUM") as base_pool,
            tc.tile_pool(name="bank_ps", bufs=6, space="PSUM") as bank_pool,
            tc.tile_pool(name="bs_sb", bufs=3) as bs_pool,
            tc.tile_pool(name="s_sb", bufs=6) as s_pool,
            tc.tile_pool(name="r_sb2", bufs=3) as rr_pool,
        ):
            tiles = [(fc, tt) for fc in range(FCH) for tt in range(NTT)]
            pend = {}
            wi_cur = lb_cur = None
            for i in range(len(tiles) + 1):
                if i < len(tiles):
                    fc, tt = tiles[i]
                    tsl = slice(tt * TT, (tt + 1) * TT)
                    if tt == 0:
                        wi_cur = wi_pool.tile([128, DCH, 128], BF16, tag="wi",
                                              name=f"wi{fc}")
                        nc.sync.dma_start(out=wi_cur, in_=wiB[fc])
                        lb_cur = lb_pool.tile([128, 128], BF16, tag="lb",
                                              name=f"lb{fc}")
                        nc.sync.dma_start(out=lb_cur, in_=lBc[fc])
                    banks = []
                    for tA_m in (tA1_sb, tA2_sb):
                        bank = bank_pool.tile([128, TT], F32, tag="bank")
                        nc.tensor.matmul(
                            bank, lhsT=lb_cur, rhs=tA_m[:, tsl],
                            start=True, stop=(tA_m is tA2_sb),
                        )
                        banks.append(bank)
                    bps = base_pool.tile([128, TT], F32, tag="base")
                    for ci in range(DCH):
                        nc.tensor.matmul(
                            bps,
                            lhsT=wi_cur[:, ci, :],
                            rhs=hsB_sb[:, ci, tsl],
                            start=(ci == 0), stop=(ci == DCH - 1),
                        )
                    bs = bs_pool.tile([128, TT], BF16, tag="bs")
                    nc.vector.tensor_scalar(
                        out=bs, in0=bps, scalar1=bi_sb[:, fc:fc + 1],
                        scalar2=None, op0=ALU.add)
                    # choice 1: base-add via identity matmul on the PE
                    nc.tensor.matmul(
                        banks[0], lhsT=identb, rhs=bs,
                        start=False, stop=True,
                    )
                    pend[i] = (fc, tt, tsl, banks, bs)
                if i - 1 in pend:
                    fc0, tt0, tsl0, banks0, bs0 = pend.pop(i - 1)
                    # choice 1: relu on ACT; choice 2: base-add + relu on DVE
                    r1 = rr_pool.tile([128, TT], BF16, tag="r")
                    nc.scalar.activation(r1, banks0[0], AF.Relu)
                    s2 = s_pool.tile([128, TT], BF16, tag="s")
                    nc.vector.scalar_tensor_tensor(
                        out=s2, in0=banks0[1], scalar=0.0, in1=bs0,
                        op0=ALU.add, op1=ALU.add)
                    acc = rr_pool.tile([128, TT], BF16, tag="acc")
                    nc.vector.scalar_tensor_tensor(
                        out=acc, in0=s2, scalar=0.0, in1=r1,
                        op0=ALU.max, op1=ALU.add)
                    nc.vector.tensor_tensor(
                        out=H_sb[fc0][:, tsl0], in0=acc, in1=V_b[:, tsl0],
                        op=ALU.mult)

        # ---- phase 4: wo matmul (+ rank-1 2*v*bo term) ----
        with (
            tc.tile_pool(name="wo_ps", bufs=1, space="PSUM") as wo_ps,
            tc.tile_pool(name="o_sb", bufs=3) as o_pool,
        ):
            for tt in range(NTT):
                tsl = slice(tt * TT, (tt + 1) * TT)
                ops = [wo_ps.tile([128, TT], F32, tag=f"o{dc}", name=f"o{tt}_{dc}")
                       for dc in range(8)]
                for fc in range(FCH):
                    for dc in range(8):
                        nc.tensor.matmul(
                            ops[dc],
                            lhsT=wo_all[:, fc, dc * 128:(dc + 1) * 128],
                            rhs=H_sb[fc][:, tsl],
                            start=(fc == 0), stop=False,
                        )
                for dc in range(8):
                    nc.tensor.matmul(
                        ops[dc],
                        lhsT=bo2_sb[0:1, dc * 128:(dc + 1) * 128],
                        rhs=vTb_sb[0:1, tsl],
                        start=False, stop=True,
                    )
                    o_t = o_pool.tile([128, TT], F32, tag="ot")
                    nc.scalar.copy(out=o_t, in_=ops[dc])
                    nc.sync.dma_start(
                        out=outT[dc * 128:(dc + 1) * 128, tsl], in_=o_t
                    )

    nc.compile()
    return nc


def prep_inputs(hidden_states, wi, bi, wo, bo, lora_A, lora_B, gate_w, gate_b):
    """Host-side layout prep; returns per-core input maps."""
    import ml_dtypes
    bf = ml_dtypes.bfloat16
    f32 = np.float32
    hs = np.asarray(hidden_states, f32).reshape(TOK, DM)
    wi = np.asarray(wi, f32); bi = np.asarray(bi, f32)
    wo = np.asarray(wo, f32); bo = np.asarray(bo, f32)
    lora_A = np.asarray(lora_A, f32); lora_B = np.asarray(lora_B, f32)
    gate_w = np.asarray(gate_w, f32); gate_b = np.asarray(gate_b, f32)

    # wi^T in (fc, d, ci, f) bf16 chunks
    wiB = np.ascontiguousarray(
        wi.T.reshape(DCH, 128, FCH, 128).transpose(2, 1, 0, 3)).astype(bf)
    biC = np.ascontiguousarray(bi.reshape(FCH, 128).T)

    # lora-A concatenated: columns 16e+r = lora_A[e,r,:]
    lA_cat = np.concatenate([lora_A[e].T for e in range(E)], axis=1)  # [DM,128]
    lAc = np.ascontiguousarray(lA_cat.reshape(DCH, 128, 128).transpose(1, 0, 2)
                               ).astype(bf)
    # lora-B concatenated: rows 16e+r = lora_B[e,:,r]
    lB_cat = np.concatenate([lora_B[e].T for e in range(E)], axis=0)  # [128,FF]
    lBc = np.ascontiguousarray(
        lB_cat.reshape(128, FCH, 128).transpose(1, 0, 2)).astype(bf)

    KD = NKC * 128
    gwA = np.zeros((KD, E), f32)
    gwA[:DM] = gate_w.T
    gwA[DM] = gate_b
    gw9 = np.ascontiguousarray(gwA.reshape(NKC, 128, E).transpose(1, 0, 2))

    woB = np.ascontiguousarray(wo.T.reshape(FCH, 128, DM)).astype(bf)
    bo2 = (2.0 * bo).astype(bf).reshape(1, DM)

    shared = dict(wiB=wiB, biC=biC, lAc=lAc, lBc=lBc, gw9=gw9, woB=woB, bo2=bo2)
    in_maps = []
    for c in range(NCORES):
        hsc = hs[c * T:(c + 1) * T]
        hsA = np.zeros((KD, T), f32)
        hsA[:DM] = hsc.T
        hsA[DM] = 1.0
        hsR = np.ascontiguousarray(hsA.reshape(NKC, 128, T).transpose(1, 0, 2))
        hsB = np.ascontiguousarray(
            hsc.T.reshape(DCH, 128, T).transpose(1, 0, 2)).astype(bf)
        in_maps.append(dict(hsR=hsR, hsB=hsB, **shared))
    return in_maps


def run(in_maps, **kwargs):
    nc = build_bass()
    return nc, run_bass_kernel_spmd(nc, in_maps, list(range(NCORES)), **kwargs)


def kernel(hidden_states, wi, bi, wo, bo, lora_A, lora_B, gate_w, gate_b):
    in_maps = prep_inputs(hidden_states, wi, bi, wo, bo, lora_A, lora_B,
                          gate_w, gate_b)
    _, res = run(in_maps)
    out = np.stack([res.results[c]["outT"].T for c in range(NCORES)])
    return out.reshape(B, S, DM).astype(np.float32)
